# revision 112
# baseline (speedup 1.0000x reference)
"""nn_CrossAttention Trainium2 kernel — 8-core data-parallel over batch.

Per core (batch slice b=1):
  - bf16 1x1 convs in transposed orientation (stationary = input slab,
    moving = weights), PSUM banks rotated over two pools to hide the
    copy+semaphore chain.
  - depthwise 3x3/7x7 as per-(channel, dh) banded-Toeplitz matmuls:
    T tables partition-major in DRAM (128 large descriptors per wave),
    float8 e3m4 with a x16 pre-scale (cancels in the q/k l2-norms,
    divided out of wp for the v path), wave DMAs issued from the Pool
    engine's SWDGE so their waits stay off the SP queue, 4 channels per
    PSUM bank with a single batched writeback copy.
  - l2 norms via PE gram matrices (q^T q, k^T k accumulated like QK^T);
    diagonals extracted with an identity-mask multiply + ones matmul.
  - QK^T with w on partitions; softmax without max-subtraction
    (|logits| <= temperature); attn@v merged with the output 1x1 conv
    through a per-pair [96,192] fused matrix; bf16 output staging.
  - Phase pipeline: A(q) -> [A(kv) || dw(q)] -> [dw(k) || QK(p0) ||
    grams] -> [dw(v,p0) || QK(p1) || gram-k(p1)] -> [dw(v,p1) ||
    vtrans(p0) || D(p0)] -> D(p1) -> [vtrans(p1) || out(rows 0-128)]
    -> out(rows 128-192).
  PSUM->SBUF writebacks alternate DVE (2/3) and ACT Copy (1/3).
"""

import sys

sys.path.insert(0, "/opt/trn_rl_repo")

import numpy as np
import ml_dtypes

B, C, Himg, Wimg = 8, 192, 128, 128
HW = Himg * Wimg
HEADS, DHC = 4, 48      # heads, channels per head
PC = 96                 # channels per head-pair
SLAB = 8                # image rows per input stream slab
Q_CW = 8                # channels per T-wave (3x3)
KV_CW = 4               # channels per T-wave (7x7)

_PROG = None            # cached program


def _build_toeplitz_pm(wdw, ksz):
    """wdw [c, ksz, ksz] f32 -> [128, c*ksz, 128] bf16, partition-major.

    T[p=w_in, c*ksz+dh, w_out] = wdw[c, dh, w_in - w_out + pad] in band.
    """
    pad = ksz // 2
    wi = np.arange(128)[:, None]
    wo = np.arange(128)[None, :]
    idx = wi - wo + pad
    valid = (idx >= 0) & (idx < ksz)
    idxc = np.clip(idx, 0, ksz - 1)
    T = wdw[:, :, idxc] * valid[None, None]        # [c, ksz, 128(p), 128(wo)]
    # x16 pre-scale puts the weights in e3m4's normal range (4 mantissa
    # bits); the scale cancels in the q/k l2-norms and is divided back out
    # of the host-side wp for the v path.
    T_pm = np.ascontiguousarray(T.transpose(2, 0, 1, 3)) * 16.0
    c = wdw.shape[0]
    T_pm = np.clip(T_pm, -15.5, 15.5)
    return T_pm.reshape(128, c * ksz, 128).astype(ml_dtypes.float8_e3m4)


def _split_excess_waits(nc, limit=1):
    """This container's walrus rejects >1 sync wait per instruction (and any
    wait on Drain beyond its own barrier). Hoist extras onto same-engine
    NoOps placed immediately before."""
    import bass_rust
    import concourse.mybir as mybir

    n_split = 0
    for fn in nc.m.functions:
        for bb in fn.blocks:
            insts = bb.instructions
            i = 0
            while i < len(insts):
                inst = insts[i]
                si = inst.sync_info
                lim = 0 if type(inst).__name__ == "InstDrain" else limit
                if si is not None and si.on_wait and len(si.on_wait) > lim:
                    waits = list(si.on_wait)
                    keep, extra = waits[:lim], waits[lim:]
                    pos = i
                    for j in range(0, len(extra), max(limit, 1)):
                        ch = extra[j : j + max(limit, 1)]
                        nop = mybir.InstNoOp(
                            name=f"waitsplit_{n_split}_{pos}",
                            engine=inst.engine,
                            ins=[],
                            outs=[],
                            sync_info=bass_rust.SyncInfo(on_wait=ch, on_update=[]),
                        )
                        insts.insert(pos, nop)
                        pos += 1
                        n_split += 1
                    inst.sync_info = bass_rust.SyncInfo(
                        on_wait=keep, on_update=list(si.on_update)
                    )
                    i = pos + 1
                else:
                    i += 1
    return n_split


def _build_program():
    import contextlib

    import concourse.bass as bass
    import concourse.mybir as mybir
    import concourse.tile as tile

    F32 = mybir.dt.float32
    BF16 = mybir.dt.bfloat16
    F8 = mybir.dt.float8e3
    AF = mybir.ActivationFunctionType
    OP = mybir.AluOpType

    nc = bass.Bass("TRN2", target_bir_lowering=False, debug=False, num_devices=8)

    # ---- DRAM parameters ----
    xin = nc.dram_tensor("x", [C, HW], BF16, kind="ExternalInput").ap()
    yin = nc.dram_tensor("y", [C, HW], BF16, kind="ExternalInput").ap()
    wq_d = nc.dram_tensor("wq", [C, C], BF16, kind="ExternalInput").ap()
    wkv_d = nc.dram_tensor("wkv", [C, 2 * C], BF16, kind="ExternalInput").ap()
    wp_d = nc.dram_tensor("wp", [2, PC, C], BF16, kind="ExternalInput").ap()
    tq_d = nc.dram_tensor("tq", [128, C * 3, 128], F8, kind="ExternalInput").ap()
    tkv_d = nc.dram_tensor(
        "tkv", [128, 2 * C * 7, 128], F8, kind="ExternalInput"
    ).ap()
    idb_d = nc.dram_tensor("idb", [128, 128], BF16, kind="ExternalInput").ap()
    mask_d = nc.dram_tensor("maskbd", [PC, PC], F32, kind="ExternalInput").ap()
    idm_d = nc.dram_tensor("idm", [PC, PC], F32, kind="ExternalInput").ap()
    temp_d = nc.dram_tensor("temprow", [1, C], F32, kind="ExternalInput").ap()
    out_d = nc.dram_tensor("out", [C, HW], BF16, kind="ExternalOutput").ap()

    with tile.TileContext(nc) as tc:
        with contextlib.ExitStack() as ctx:
            consts = ctx.enter_context(tc.tile_pool(name="consts", bufs=1))
            s1 = ctx.enter_context(tc.tile_pool(name="s1", bufs=1))
            s2 = ctx.enter_context(tc.tile_pool(name="s2", bufs=1))
            streams = ctx.enter_context(tc.tile_pool(name="streams", bufs=3))
            tpool = ctx.enter_context(tc.tile_pool(name="tpool", bufs=5))
            psA = ctx.enter_context(tc.tile_pool(name="psA", bufs=2, space="PSUM"))
            psdw = ctx.enter_context(tc.tile_pool(name="psdw", bufs=2, space="PSUM"))
            pqk = ctx.enter_context(tc.tile_pool(name="pqk", bufs=1, space="PSUM"))
            pst = ctx.enter_context(tc.tile_pool(name="pst", bufs=2, space="PSUM"))
            ostage = ctx.enter_context(tc.tile_pool(name="ostage", bufs=4))
            misc = ctx.enter_context(tc.tile_pool(name="misc", bufs=1))
            stats = ctx.enter_context(tc.tile_pool(name="stats", bufs=1))

            # ---- load constants ----
            wq0 = consts.tile([128, C], BF16)
            wq1 = consts.tile([64, C], BF16)
            nc.sync.dma_start(out=wq0, in_=wq_d[0:128, :])
            nc.sync.dma_start(out=wq1, in_=wq_d[128:192, :])
            wkv0 = consts.tile([128, 2 * C], BF16)
            wkv1 = consts.tile([64, 2 * C], BF16)
            wp0 = consts.tile([PC, C], BF16)
            wp1 = consts.tile([PC, C], BF16)
            identb = consts.tile([128, 128], BF16)
            maskbd = consts.tile([PC, PC], F32)
            idmask = consts.tile([PC, PC], F32)
            temprow = consts.tile([1, C], F32)
            onescol = consts.tile([128, 1], BF16)
            ones1 = consts.tile([1, 128], BF16)

            def load_late_consts():
                # issued after phase A starts; needed only from W3 onward
                nc.sync.dma_start(out=wkv0, in_=wkv_d[0:128, :])
                nc.sync.dma_start(out=wkv1, in_=wkv_d[128:192, :])
                nc.sync.dma_start(out=wp0, in_=wp_d[0])
                nc.sync.dma_start(out=wp1, in_=wp_d[1])
                nc.sync.dma_start(out=identb, in_=idb_d)
                nc.sync.dma_start(out=maskbd, in_=mask_d)
                nc.sync.dma_start(out=idmask, in_=idm_d)
                nc.sync.dma_start(out=temprow, in_=temp_d)
                nc.vector.memset(onescol, 1.0)
                nc.vector.memset(ones1, 1.0)

            # ---- big SBUF regions ----
            # bq: [w, h*C + c] bf16; bkv: [w, h*2C + c] (k: c<192, v: c>=192)
            bq = s1.tile([128, Himg * C], BF16, tag="qv")
            bkv = s2.tile([128, Himg * 2 * C], BF16, tag="kv")
            bq3 = bq.rearrange("p (h c) -> p h c", c=C)
            bq_cf = bq.rearrange("p (h c) -> p c h", c=C)
            bkv3 = bkv.rearrange("p (h c) -> p h c", c=2 * C)
            bkv_cf = bkv.rearrange("p (h c) -> p c h", c=2 * C)

            # PSUM->SBUF writebacks: DVE 2 of 3, ACT (Copy) 1 of 3.
            cp_state = [0]

            def wb_copy(out, in_, act_mod=3):
                i = cp_state[0] = cp_state[0] + 1
                if i % act_mod == 0:
                    nc.scalar.activation(out=out, in_=in_, func=AF.Copy)
                else:
                    nc.vector.tensor_copy(out, in_)

            # ============ phase A: 1x1 convs (transposed orientation) =======
            def gen_conv1x1(src_d, mov0, mov1, nmov, wb, per_bank,
                            borrow_dw=False):
                # wb(h0, pa): write back per_bank h-columns from psum pa
                pa = [None]
                for h in range(Himg):
                    sl = h % SLAB
                    if sl == 0:
                        xs0 = streams.tile([128, SLAB * 128], BF16, tag="st0")
                        xs1 = streams.tile([64, SLAB * 128], BF16, tag="st1")
                        nc.sync.dma_start(
                            out=xs0, in_=src_d[0:128, h * 128 : (h + SLAB) * 128]
                        )
                        nc.sync.dma_start(
                            out=xs1, in_=src_d[128:192, h * 128 : (h + SLAB) * 128]
                        )
                    hb = h % per_bank
                    if hb == 0:
                        # 6-bank rotation (psA + pst + pqk's pre-W3-idle
                        # slots) hides the copy+sem chain
                        rot = (
                            (psA, "pA"), (pst, "tp"), (pqk, "at"),
                            (psA, "pA"), (pst, "tp"), (pqk, "gk"),
                        )
                        if borrow_dw:
                            rot = rot + ((psdw, "pdw"), (psdw, "pdw"))
                        bpool, btag = rot[(h // per_bank) % len(rot)]
                        pa[0] = bpool.tile([128, 512], F32, tag=btag, name=f"pa_{h}")
                    nc.tensor.matmul(
                        pa[0][:, hb * nmov : (hb + 1) * nmov],
                        xs0[:, sl * 128 : (sl + 1) * 128],
                        mov0,
                        start=True,
                        stop=False,
                    )
                    nc.tensor.matmul(
                        pa[0][:, hb * nmov : (hb + 1) * nmov],
                        xs1[:, sl * 128 : (sl + 1) * 128],
                        mov1,
                        start=False,
                        stop=True,
                    )
                    if hb == per_bank - 1:
                        wb(h - hb, pa[0])
                        yield

            # ============ phase B: depthwise via Toeplitz matmuls ===========
            def gen_dw(chan_ap, wb_view, t_dram, ksz, tch0, cw, cvals):
                # chan_ap(ci, h0, cnt) -> [128, cnt] moving view of channel ci
                # wb_view(c0, nch) -> writeback dest for channels c0..c0+nch
                pad = ksz // 2
                order = [pad] + [d for d in range(ksz) if d != pad]
                wave = [None]
                pdw = [None]
                for n, ci in enumerate(cvals):
                    if n % cw == 0:
                        nwv = min(cw, len(cvals) - n) * ksz
                        wave[0] = tpool.tile(
                            [128, KV_CW * 7, 128], F8, tag="tw",
                            name=f"tw_{tch0}_{ci}",
                        )
                        i0 = (tch0 + ci) * ksz
                        # Pool-engine SWDGE: keeps the T-stream's blocking
                        # waits off the shared SP DMA-issue queue.
                        nc.gpsimd.dma_start(
                            out=wave[0][:, 0:nwv, :],
                            in_=t_dram[:, i0 : i0 + nwv, :],
                        )
                    q = n % 4
                    if q == 0:
                        dpool, dtag = ((psdw, "pdw"), (psA, "pA"))[(n // 4) % 2]
                        pdw[0] = dpool.tile(
                            [128, 512], F32, tag=dtag, name=f"pdw_{tch0}_{ci}"
                        )
                    base = (n % cw) * ksz
                    for j, dh in enumerate(order):
                        sh = dh - pad
                        cnt = Himg - abs(sh)
                        h0o, h0i = max(0, -sh), max(0, sh)
                        nc.tensor.matmul(
                            pdw[0][:, q * 128 + h0o : q * 128 + h0o + cnt],
                            wave[0][:, base + dh, :],
                            chan_ap(ci, h0i, cnt),
                            start=(j == 0),
                            stop=(j == len(order) - 1),
                        )
                    if q == 3 or n == len(cvals) - 1:
                        wb_copy(
                            wb_view(ci - q, q + 1),
                            pdw[0][:, 0 : (q + 1) * 128],
                            act_mod=2,
                        )
                        yield

            # ============ QK^T + gram accumulation ==========================
            # attnp cols: [attn P0 | attn P1 | gram-q P0 | gram-q P1]
            # (tiles created after phase A, which borrows these tag slots)
            attnp = None
            gramk = None

            def gen_qk(P):
                for h in range(Himg):
                    nc.tensor.matmul(
                        attnp[:, P * PC : (P + 1) * PC],
                        bkv3[:, h, PC * P : PC * P + PC],
                        bq3[:, h, PC * P : PC * P + PC],
                        start=(h == 0),
                        stop=(h == Himg - 1),
                    )
                    if h % 4 == 3:
                        yield

            def gen_gram(src3, coff, P, dst):
                # dst [PC, PC] psum region <- src_slice^T @ src_slice
                for h in range(Himg):
                    sl = src3[:, h, coff + PC * P : coff + PC * P + PC]
                    nc.tensor.matmul(
                        dst, sl, sl, start=(h == 0), stop=(h == Himg - 1)
                    )
                    if h % 4 == 3:
                        yield

            # ============ v transpose -> vt [c, spatial] ====================
            vt = s1.tile([PC, 2 * HW], BF16, tag="qv")

            def gen_vtrans(P):
                for hb in range(0, Himg, 4):
                    ptv = pst.tile([PC, 512], BF16, tag="tp", name=f"ptv{P}_{hb}")
                    for j in range(4):
                        nc.tensor.transpose(
                            ptv[:, j * 128 : (j + 1) * 128],
                            bkv3[:, hb + j, C + PC * P : C + PC * P + PC],
                            identb,
                        )
                    wb_copy(vt[:, P * HW + hb * 128 : P * HW + (hb + 4) * 128], ptv)
                    yield

            # ============ drivers ===========================================
            def drain(g):
                for _ in g:
                    pass

            def interleave(*gens):
                gens = list(gens)
                while gens:
                    done = []
                    for g in gens:
                        if next(g, StopIteration) is StopIteration:
                            done.append(g)
                    for g in done:
                        gens.remove(g)

            # W1: A-q alone
            def wb_q(h0, pa):
                wb_copy(bq[:, h0 * C : (h0 + 2) * C], pa[:, 0 : 2 * C], act_mod=2)

            gq1 = gen_conv1x1(xin, wq0, wq1, C, wb_q, 2, borrow_dw=True)
            for _ in range(4):
                next(gq1, None)
            load_late_consts()
            drain(gq1)

            # W2: A-kv || dw-q
            def wb_kv(h, pa):
                wb_copy(
                    bkv[:, h * 2 * C : (h + 1) * 2 * C], pa[:, 0 : 2 * C],
                    act_mod=2,
                )

            def q_chan(ci, h0, cnt):
                return bq3[:, h0 : h0 + cnt, ci]

            def q_wb(c0, nch):
                return bq_cf[:, c0 : c0 + nch, :]

            gkv = gen_conv1x1(yin, wkv0, wkv1, 2 * C, wb_kv, 1)
            gdq = gen_dw(q_chan, q_wb, tq_d, 3, 0, Q_CW, list(range(C)))
            while True:
                a = next(gkv, StopIteration)
                next(gkv, None)
                b = next(gdq, StopIteration)
                if a is StopIteration and b is StopIteration:
                    break

            # ============ softmax prep per pair (D phase) ===================
            mps = [None, None]

            def gen_dphase(P):
                # rq as a row [1, PC]: diag(gram-q) then 1/sqrt, * temp
                gmq = misc.tile([PC, PC], BF16, tag="gmq", name=f"gmq{P}")
                nc.vector.tensor_tensor(
                    gmq, attnp[:, (2 + P) * PC : (3 + P) * PC], idmask, op=OP.mult
                )
                prow = pst.tile([1, PC], F32, tag="tp", name=f"prow{P}")
                nc.tensor.matmul(
                    prow, onescol[0:PC], gmq, start=True, stop=True
                )
                sq_row = misc.tile([1, PC], F32, tag="m1", name=f"m1_{P}")
                nc.scalar.activation(out=sq_row, in_=prow, func=AF.Sqrt)
                rq_row = misc.tile([1, PC], F32, tag="m2", name=f"m2_{P}")
                nc.vector.reciprocal(rq_row, sq_row)
                nc.vector.tensor_tensor(
                    rq_row, rq_row, temprow[:, PC * P : PC * P + PC], op=OP.mult
                )
                rq_bf = misc.tile([1, PC], BF16, tag="m3", name=f"m3_{P}")
                nc.vector.tensor_copy(rq_bf, rq_row)
                yield
                # rk as a column [PC, 1]: diag(gram-k)
                gmk = misc.tile([PC, PC], BF16, tag="gmk", name=f"gmk{P}")
                nc.vector.tensor_tensor(
                    gmk, gramk[:, P * PC : (P + 1) * PC], idmask, op=OP.mult
                )
                pcol = pst.tile([PC, 1], F32, tag="tp", name=f"pcol{P}")
                nc.tensor.matmul(
                    pcol, gmk, onescol[0:PC], start=True, stop=True
                )
                sq_col = misc.tile([PC, 1], F32, tag="m4", name=f"m4_{P}")
                nc.scalar.activation(out=sq_col, in_=pcol, func=AF.Sqrt)
                rk_col = misc.tile([PC, 1], F32, tag="m5", name=f"m5_{P}")
                nc.vector.reciprocal(rk_col, sq_col)
                # rq replicated across partitions via K=1 matmul
                prep = pst.tile([PC, PC], F32, tag="tp", name=f"prep{P}")
                nc.tensor.matmul(
                    prep, ones1[:, 0:PC], rq_bf, start=True, stop=True
                )
                rqrep = misc.tile([PC, PC], F32, tag="m6", name=f"m6_{P}")
                nc.vector.tensor_copy(rqrep, prep)
                t1 = misc.tile([PC, PC], F32, tag="m7", name=f"m7_{P}")
                nc.vector.tensor_tensor(
                    t1, attnp[:, P * PC : (P + 1) * PC], rqrep, op=OP.mult
                )
                yield
                # exp(rk * t1), then zero junk blocks, bf16
                e1 = misc.tile([PC, PC], F32, tag="m8", name=f"m8_{P}")
                nc.scalar.activation(out=e1, in_=t1, func=AF.Exp, scale=rk_col)
                ezero = stats.tile([PC, PC], BF16, tag=f"ez{P}")
                nc.vector.tensor_tensor(ezero, e1, maskbd, op=OP.mult)
                # column sums -> recip
                pcs = pst.tile([PC, 1], F32, tag="tp", name=f"pcs{P}")
                nc.tensor.matmul(pcs, ezero, onescol[0:PC], start=True, stop=True)
                recip = stats.tile([PC, 1], F32, tag=f"rc{P}")
                nc.vector.reciprocal(recip, pcs)
                yield
                # fused (attn^T * recip) @ (wp scaled): mp [PC, C]
                ezt_ps = pst.tile([PC, PC], BF16, tag="tp", name=f"ezt_ps{P}")
                nc.tensor.transpose(ezt_ps, ezero, identb[0:PC, 0:PC])
                ezt = misc.tile([PC, PC], BF16, tag="m9", name=f"m9_{P}")
                nc.vector.tensor_copy(ezt, ezt_ps)
                wsc = misc.tile([PC, C], BF16, tag="m10", name=f"m10_{P}")
                nc.vector.tensor_scalar_mul(wsc, (wp0, wp1)[P], recip)
                pmp = pst.tile([PC, C], F32, tag="tp", name=f"pmp{P}")
                nc.tensor.matmul(pmp, ezt, wsc, start=True, stop=True)
                mp = stats.tile([PC, C], BF16, tag=f"mp{P}")
                nc.vector.tensor_copy(mp, pmp)
                mps[P] = mp
                yield

            # ============ G: fused (attn @ v) + proj -> out =================
            def gen_gout(mi):
                r0, r1 = ((0, 128), (128, 192))[mi]
                mw = r1 - r0
                so = [None]
                for ki, n in enumerate(range(0, HW, 512)):
                    if ki % 2 == 0:
                        so[0] = ostage.tile(
                            [128, 1024], BF16, tag="os", name=f"so{mi}_{ki}"
                        )
                    pool, tg = (
                        (psA, "pA"), (psdw, "pdw"), (pqk, "at"),
                        (psA, "pA"), (psdw, "pdw"), (pqk, "gk"),
                    )[ki % 6]
                    po = pool.tile([128, 512], F32, tag=tg, name=f"po{mi}_{ki}")
                    nc.tensor.matmul(
                        po[0:mw], mps[0][:, r0:r1], vt[:, n : n + 512],
                        start=True, stop=False,
                    )
                    nc.tensor.matmul(
                        po[0:mw], mps[1][:, r0:r1], vt[:, HW + n : HW + n + 512],
                        start=False, stop=True,
                    )
                    half = (ki % 2) * 512
                    if ki % 2 == 0:
                        nc.vector.tensor_copy(
                            so[0][0:mw, half : half + 512], po[0:mw]
                        )
                    else:
                        nc.scalar.activation(
                            out=so[0][0:mw, half : half + 512],
                            in_=po[0:mw],
                            func=AF.Copy,
                        )
                        nc.sync.dma_start(
                            out=out_d[r0:r1, n - 512 : n + 512],
                            in_=so[0][0:mw],
                        )
                    yield

            def k_chan(ci, h0, cnt):
                return bkv3[:, h0 : h0 + cnt, ci]

            def k_wb(c0, nch):
                return bkv_cf[:, c0 : c0 + nch, :]

            def v_chan(ci, h0, cnt):
                return bkv3[:, h0 : h0 + cnt, C + ci]

            def v_wb(c0, nch):
                return bkv_cf[:, C + c0 : C + c0 + nch, :]

            attnp = pqk.tile([PC, 4 * PC], F32, tag="at")
            gramk = pqk.tile([PC, 2 * PC], F32, tag="gk")

            # W3: dw-k first half; then second half || QK(P0) || grams
            drain(gen_dw(k_chan, k_wb, tkv_d, 7, 0, KV_CW, list(range(0, PC))))
            interleave(
                gen_dw(k_chan, k_wb, tkv_d, 7, 0, KV_CW, list(range(PC, C))),
                gen_qk(0),
                gen_gram(bq3, 0, 0, attnp[:, 2 * PC : 3 * PC]),
                gen_gram(bq3, 0, 1, attnp[:, 3 * PC : 4 * PC]),
                gen_gram(bkv3, 0, 0, gramk[:, 0:PC]),
            )

            # W4a: dw-v (pair0) || QK(P1) || gram-k(P1)
            interleave(
                gen_dw(v_chan, v_wb, tkv_d, 7, C, KV_CW, list(range(0, PC))),
                gen_qk(1),
                gen_gram(bkv3, 0, 1, gramk[:, PC : 2 * PC]),
            )

            # W4b: dw-v (pair1) || vtrans(P0) || D(P0) || D(P1)
            gdv = gen_dw(v_chan, v_wb, tkv_d, 7, C, KV_CW, list(range(PC, C)))
            gvt = gen_vtrans(0)
            gd0 = gen_dphase(0)
            gd1 = gen_dphase(1)
            while True:
                a = next(gdv, StopIteration)
                b = next(gvt, StopIteration)
                next(gvt, None)
                c = next(gd0, StopIteration)
                d = next(gd1, StopIteration)
                if all(x is StopIteration for x in (a, b, c, d)):
                    break

            # W5: vtrans(P1) || G(rows 0-128) || G(rows 128-192)
            gvt1 = gen_vtrans(1)
            gg0 = gen_gout(0)
            gg1 = gen_gout(1)
            while True:
                a = next(gvt1, StopIteration)
                next(gvt1, None)
                b = next(gg0, StopIteration)
                c = next(gg1, StopIteration)
                if all(x is StopIteration for x in (a, b, c)):
                    break

    _split_excess_waits(nc)
    return nc


def _get_program():
    global _PROG
    if _PROG is None:
        _PROG = _build_program()
    return _PROG


def kernel(x, y, q_w, q_dw_w, kv_w, kv_dw_w, proj_w, temperature):
    return _run(x, y, q_w, q_dw_w, kv_w, kv_dw_w, proj_w, temperature)[0]


def _run(x, y, q_w, q_dw_w, kv_w, kv_dw_w, proj_w, temperature, trace=False):
    from concourse.bass_utils import run_bass_kernel_spmd

    BF = ml_dtypes.bfloat16
    x = np.asarray(x, dtype=np.float32).reshape(B, C, HW).astype(BF)
    y = np.asarray(y, dtype=np.float32).reshape(B, C, HW).astype(BF)
    q_w = np.asarray(q_w, dtype=np.float32)
    kv_w = np.asarray(kv_w, dtype=np.float32)
    proj_w = np.asarray(proj_w, dtype=np.float32)
    q_dw_w = np.asarray(q_dw_w, dtype=np.float32)
    kv_dw_w = np.asarray(kv_dw_w, dtype=np.float32)
    temperature = np.asarray(temperature, dtype=np.float32).reshape(HEADS)

    wq = np.ascontiguousarray(q_w[:, :, 0, 0].T).astype(BF)     # [C, C]
    wkv = np.ascontiguousarray(kv_w[:, :, 0, 0].T).astype(BF)   # [C, 2C]
    wpT = proj_w[:, :, 0, 0].T / 16.0   # /16 undoes the e3m4 T pre-scale on v
    wp = np.stack([wpT[0:PC], wpT[PC:C]]).astype(BF)
    tq = _build_toeplitz_pm(q_dw_w[:, 0], 3)
    tkv = _build_toeplitz_pm(kv_dw_w[:, 0], 7)
    idb = np.eye(128, dtype=BF)
    maskbd = np.zeros((PC, PC), np.float32)
    maskbd[0:DHC, 0:DHC] = 1.0
    maskbd[DHC:PC, DHC:PC] = 1.0
    idm = np.eye(PC, dtype=np.float32)
    temprow = np.repeat(temperature, DHC).reshape(1, C)

    shared = {
        "wq": wq, "wkv": wkv, "wp": wp, "tq": tq, "tkv": tkv,
        "idb": idb, "maskbd": maskbd, "idm": idm, "temprow": temprow,
    }
    in_maps = [dict(shared, x=x[i], y=y[i]) for i in range(B)]

    nc = _get_program()
    res = run_bass_kernel_spmd(
        nc, in_maps, core_ids=list(range(B)), trace=trace
    )
    out = np.stack([res.results[i]["out"] for i in range(B)])
    return out.reshape(B, C, Himg, Wimg).astype(np.float32), res
